# revision 6
# baseline (speedup 1.0000x reference)
"""Trainium2 Bass kernel for ChunkCausalDepthwiseConv1d (bf16 pipeline).

Problem: x (16, 512, 4096) f32; per-channel depthwise convs:
  out = chunk_scale * (chunkconv_K31_same_per_256chunk(x) + chunk_b)
        + causalconv_K16(x) + causal_b

Strategy (8 NeuronCores, channel-parallel, 64 ch/core, all batches),
everything bf16 on device (tolerance 2e-2 rel L2; bf16 lands ~3e-3):
  - host converts x to bf16 and packs per-channel Toeplitz stationaries
    (A_e/A_o within-block taps with chunk_scale folded, B next-block
    lookahead, C_e/C_o prev-block carries) in bf16, biases fp32.
  - x is loaded TIME-MAJOR directly via xbar transpose-DMAs (16 per
    channel, one per batch: DRAM [32,128] contiguous 8 KiB -> SBUF
    [128,32]), eliminating the on-chip input transposes entirely.
  - xtm layout: col = b*36 + q9*4 + j (b batch, q9 = 512-sample segment
    + 1, j = 128-block within segment); q9=0 cols are zeros (causal
    carry for t<0), memset once into two persistent buffers alternated
    by channel parity.
  - 5 conv matmuls (bf16, fp32 PSUM) per channel into psum_e/psum_o
    [128, 256], cols ordered (j2, b, q) = (u%2, b, u//2):
      ps_e: A_e @ even-blocks + B @ odd-heads + C_e @ prev-odd-tails
      ps_o: A_o @ odd-blocks + C_o @ even-tails
    where the prev-block rhs is just the even AP shifted by -1 col
    (q9=0 supplies zeros at segment starts).
  - 2 ACT Identity+bias evacuations (fp32 bias) -> otm bf16.
  - 4 output transposes as REGULAR matmuls (lhsT = otm block, rhs =
    identity) into one [128,512] psum bank -> 1 DVE copy -> out DMA
    bf16. Regular-matmul transposes pipeline as LDW+MM (~81ns) vs
    ~275ns for transpose-mode, and keep the PE HAM clock warm.
  Host converts the bf16 output back to fp32.
"""

import numpy as np
import ml_dtypes

B, C, T = 16, 512, 4096
NCORES = 8
NCH = C // NCORES          # 64 channels per core
NBLK = T // 128            # 32 blocks of 128 per batch
NU = T // 256              # 16 chunks per batch
PACKW = 392                # cols: A_e 128 | A_o 128 | {B rows0:32 | C_e,C_o rows96:128} 128 | pad

BF16 = ml_dtypes.bfloat16


def _pack_weights(causal_w, causal_b, chunk_w, chunk_b, conv_scale):
    """Build (C, 128, PACKW) bf16 stationary operands + (C, 128, 2) f32 biases."""
    w1 = np.asarray(causal_w, np.float32)[:, 0, :]     # (C,16)
    b1 = np.asarray(causal_b, np.float32)              # (C,)
    w2 = np.asarray(chunk_w, np.float32)[:, 0, :]      # (C,31)
    b2 = np.asarray(chunk_b, np.float32)               # (C,)
    cs = np.asarray(conv_scale, np.float32)            # (2,C,31)

    scale = np.ones((C, 256), np.float32)
    scale[:, :31] += cs[0]
    scale[:, 225:] += cs[1]

    k = np.arange(128)[:, None]
    m = np.arange(128)[None, :]
    d = k - m + 15
    band2 = (d >= 0) & (d <= 30)
    band1 = (d >= 0) & (d <= 15)
    d2 = np.clip(d, 0, 30)
    d1 = np.clip(d, 0, 15)

    w2d = w2[:, d2] * band2            # (C,128,128)
    w1d = w1[:, d1] * band1
    A_e = w2d * scale[:, None, :128] + w1d
    A_o = w2d * scale[:, None, 128:] + w1d

    kc = np.arange(32)[:, None]
    mc = np.arange(32)[None, :]
    kB = np.arange(32)[:, None]
    mB_ = np.arange(128)[None, :]
    dB = kB + 143 - mB_                # B (next-block -> even): w2 taps, k<15, m>=113
    mB = (dB >= 0) & (dB <= 30)
    Bw = w2[:, np.clip(dB, 0, 30)] * mB            # (C,32,128)
    dC = kc - mc - 17                  # C corners: taps 0..15 / 0..14
    mC1 = (dC >= 0) & (dC <= 15)
    Ce_t = w1[:, np.clip(dC, 0, 15)] * mC1
    Co_t = (w2[:, np.clip(dC, 0, 30)] + w1[:, np.clip(dC, 0, 15)]) * mC1

    pack = np.zeros((C, 128, PACKW), np.float32)
    pack[:, :, 0:128] = A_e
    pack[:, :, 128:256] = A_o
    # B lives in rows 0:32 of cols 256:384 (nonzero only cols 369:384);
    # C_e / C_o live in rows 96:128 of cols 256:288 / 288:320 (disjoint rows).
    pack[:, 0:32, 256:384] = Bw
    pack[:, 96:128, 256:288] = Ce_t
    pack[:, 96:128, 288:320] = Co_t

    bias = np.empty((C, 128, 2), np.float32)
    bias[:, :, 0] = scale[:, :128] * b2[:, None] + b1[:, None]   # bias_e
    bias[:, :, 1] = scale[:, 128:] * b2[:, None] + b1[:, None]   # bias_o
    return pack.astype(BF16), bias


def build_nc(nch=NCH, enable_asserts=False, loop_reps=1, skip=()):
    """Build the per-core Bass program (same NEFF for all cores)."""
    import concourse.bacc as bacc
    import concourse.mybir as mybir
    import concourse.tile as tile
    from concourse.ap import AP as BassAP

    fp32 = mybir.dt.float32
    bf16 = mybir.dt.bfloat16
    COPY = mybir.ActivationFunctionType.Identity

    nc = bacc.Bacc("TRN2", target_bir_lowering=False, debug=False,
                   enable_asserts=enable_asserts)

    x_d = nc.dram_tensor("x", [B, nch, T], bf16, kind="ExternalInput").ap()
    w_d = nc.dram_tensor("wpack", [nch, 128, PACKW], bf16, kind="ExternalInput").ap()
    bias_d = nc.dram_tensor("bias", [nch, 128, 2], fp32, kind="ExternalInput").ap()
    id_d = nc.dram_tensor("ident", [128, 128], bf16, kind="ExternalInput").ap()
    o_d = nc.dram_tensor("out", [B, nch, T], bf16, kind="ExternalOutput").ap()

    # DRAM views: x as per-(channel,batch) [32 rows, 128] for xbar transpose
    # loads; out natural per channel [b16, q8, t512] <-> sbuf [128=(b,q), 512].
    x_v = x_d.rearrange("b c (r v) -> c b r v", v=128)
    o_v = o_d.rearrange("b c (q t) -> c b q t", q=8)
    # weights: chunks of channels
    WCH = min(8, nch)  # channels per weight DMA
    w_v = w_d.rearrange("(cc c) p w -> cc p c w", cc=nch // WCH)
    bias_v = bias_d.rearrange("c p two -> p c two")

    with tile.TileContext(nc) as tc:
        with (
            tc.tile_pool(name="wbuf", bufs=1) as wbuf_pool,
            tc.tile_pool(name="ident", bufs=1) as id_pool,
            tc.tile_pool(name="xtm", bufs=1) as xtm_pool,
            tc.tile_pool(name="otm", bufs=3) as otm_pool,
            tc.tile_pool(name="onat", bufs=3) as onat_pool,
            tc.tile_pool(name="ps_conv", bufs=3, space="PSUM") as psconv_pool,
            tc.tile_pool(name="ps_ot", bufs=2, space="PSUM") as psot_pool,
        ):
            wbuf = wbuf_pool.tile([128, nch, PACKW], bf16)
            biasbuf = wbuf_pool.tile([128, nch, 2], fp32, tag="biasbuf")
            ident = id_pool.tile([128, 128], bf16)
            nc.gpsimd.dma_start(ident[:], id_d)
            nc.gpsimd.dma_start(biasbuf[:], bias_v)
            for i in range(nch // WCH):
                nc.gpsimd.dma_start(wbuf[:, i * WCH:(i + 1) * WCH, :], w_v[i])

            # Two persistent xtm buffers (alternated by channel parity) so the
            # zero columns are memset once, not per channel.
            # xtm col = b*48 + 16 + r, r = 4q + j: batch b, 512-sample segment
            # q, 128-block j within segment; cols b*48+0:16 are zero (causal
            # carry for t<0). The +16 keeps every xbar-transpose DMA dest
            # 16-col aligned (hard xbar requirement).
            xtm0 = xtm_pool.tile([128, 768], bf16, tag="xtm0", name="xtm0")
            xtm1 = xtm_pool.tile([128, 768], bf16, tag="xtm1", name="xtm1")
            xtms = [xtm0, xtm1]
            for xt in xtms:
                xtv = xt[:].rearrange("p (b w) -> p b w", b=16)
                nc.vector.memset(xtv[:, :, 0:16], 0.0)

            import contextlib
            loop_cm = (tc.For_i(0, loop_reps, 1) if loop_reps > 1
                       else contextlib.nullcontext())
            with loop_cm:
              for c in range(nch):
                  # --- load x time-major via xbar transpose DMAs ---
                  xtm = xtms[c % 2]
                  xt = xtm[:]
                  if "indma" not in skip:
                      for b in range(16):
                          nc.sync.dma_start_transpose(
                              xt[:, b * 48 + 16:b * 48 + 48], x_v[c, b])

                  # --- conv matmuls (bf16) ---
                  # psum cols (j2, b, q); rhs col = b*48 + 16 + 4q + 2*j2 + s
                  # with shift s: even blocks s=0, odd s=+1, prev block s=-1
                  # (the zero cols supply the t<0 causal carry at b*48+15).
                  def rhs(offset, parts=128):
                      return BassAP(tensor=xt.tensor, offset=offset,
                                    ap=[[768, parts], [2, 2], [48, 16], [4, 8]])
                  rhs_even = rhs(16)
                  rhs_odd = rhs(17)
                  rhs_odd_head = rhs(17, 32)
                  rhs_prev = rhs(15)
                  wA_e = wbuf[:, c, 0:128]
                  wA_o = wbuf[:, c, 128:256]
                  wB = wbuf[0:32, c, 256:384]
                  wCe = wbuf[:, c, 256:288]
                  wCo = wbuf[:, c, 288:320]

                  ps_e = psconv_pool.tile([128, 256], fp32, tag="ps_conv")
                  ps_o = psconv_pool.tile([128, 256], fp32, tag="ps_conv")
                  if "conv" not in skip:
                    nc.tensor.matmul(ps_e[:], wA_e, rhs_even,
                                     start=True, stop=False, skip_group_check=True)
                    nc.tensor.matmul(ps_e[:], wB, rhs_odd_head,
                                     start=False, stop=False, skip_group_check=True)
                    nc.tensor.matmul(ps_e[0:32, :], wCe, rhs_prev,
                                     start=False, stop=True, skip_group_check=True)

                    nc.tensor.matmul(ps_o[:], wA_o, rhs_odd,
                                     start=True, stop=False, skip_group_check=True)
                    nc.tensor.matmul(ps_o[0:32, :], wCo, rhs_even,
                                     start=False, stop=True, skip_group_check=True)

                  # --- evacuate with bias (ACT) ---
                  otm_e = otm_pool.tile([128, 256], bf16, tag="otm")
                  otm_o = otm_pool.tile([128, 256], bf16, tag="otm")
                  if "act" not in skip:
                      nc.scalar.activation(otm_e[:], ps_e[:], COPY,
                                           bias=biasbuf[:, c, 0:1])
                      nc.scalar.activation(otm_o[:], ps_o[:], COPY,
                                           bias=biasbuf[:, c, 1:2])

                  # --- transpose back to natural [128=(b,q), 512t], store ---
                  onat = onat_pool.tile([128, 512], bf16, tag="onat")
                  ps_ot = psot_pool.tile([128, 512], fp32, tag="ps_ot")
                  if "outtrans" not in skip:
                      for half in range(2):
                          for par, otm in ((0, otm_e), (1, otm_o)):
                              nc.tensor.matmul(
                                  ps_ot[:, half * 256 + par * 128:
                                        half * 256 + (par + 1) * 128],
                                  otm[:, half * 128:(half + 1) * 128],
                                  ident[:],
                                  start=True, stop=True, skip_group_check=True)
                      if "outevac" not in skip:
                          nc.vector.tensor_copy(onat[:], ps_ot[:])
                  if "outdma" not in skip:
                      nc.scalar.dma_start(o_v[c], onat[:])

    nc.compile()
    return nc


def make_core_inputs(x, causal_w, causal_b, chunk_w, chunk_b, conv_scale):
    """Shard host-side inputs for the 8 cores (bf16 conversion included)."""
    x = np.asarray(x, np.float32).astype(BF16)
    pack, bias = _pack_weights(causal_w, causal_b, chunk_w, chunk_b, conv_scale)
    ident = np.eye(128, dtype=BF16)
    in_maps = []
    for i in range(NCORES):
        sl = slice(i * NCH, (i + 1) * NCH)
        in_maps.append({
            "x": np.ascontiguousarray(x[:, sl, :]),
            "wpack": np.ascontiguousarray(pack[sl]),
            "bias": np.ascontiguousarray(bias[sl]),
            "ident": ident,
        })
    return in_maps


def kernel(x, causal_w, causal_b, chunk_w, chunk_b, conv_scale, chunk_size):
    from concourse.bass_utils import run_bass_kernel_spmd

    assert int(chunk_size) == 256
    in_maps = make_core_inputs(x, causal_w, causal_b, chunk_w, chunk_b,
                               conv_scale)
    nc = build_nc()
    core_ids = list(range(NCORES))
    res = run_bass_kernel_spmd(nc, in_maps, core_ids)
    out = np.empty((B, C, T), np.float32)
    for i in core_ids:
        out[:, i * NCH:(i + 1) * NCH, :] = res.results[i]["out"].astype(np.float32)
    return out


# revision 7
# speedup vs baseline: 7.9464x; 7.9464x over previous
"""Trainium2 Bass kernel for ChunkCausalDepthwiseConv1d (bf16, SW-pipelined).

Problem: x (16, 512, 4096) f32; per-channel depthwise convs:
  out = chunk_scale * (chunkconv_K31_same_per_256chunk(x) + chunk_b)
        + causalconv_K16(x) + causal_b

Strategy (8 NeuronCores, channel-parallel, 64 ch/core, all batches),
everything bf16 on device (tolerance 2e-2 rel L2; bf16 lands ~3e-3):
  Per-channel Toeplitz matmuls on the TensorEngine in a time-major
  (transposed) domain. All transposes run as REGULAR matmuls (lhsT =
  data block, rhs = identity): they pipeline as LDW+MM pairs (~81ns vs
  ~275ns for transpose-mode) and keep the PE HAM clock at 2.4 GHz.
  bf16 halves DMA (x 8 MiB + wpack 6.4 MiB + out 8 MiB per core) and
  enables fast-weight-load on the 128-col stationaries.

  The 8 per-channel stages are SOFTWARE-PIPELINED across channels with
  explicit lags so every engine's in-order queue always holds ready
  work (without this, the PE serializes each channel's full
  cross-engine round trip: measured 3.1us/ch vs ~1.3us of PE work):
    lag 0: x DMA          [SP]    xnat [128=(b,q), 512t]
    lag 1: 4 in-transposes [PE] -> ps_it [128,512], 1 evac [DVE] ->
           xtm[t, j*144+b*9+q9] (q9=0 cols zero; memset hoisted into
           two persistent buffers alternated by channel parity)
    lag 2: 5 conv matmuls  [PE] -> ps_e/ps_o [128,256], cols (u%2,b,u//2)
    lag 3: 2 Identity+bias [ACT] (fp32 bias tensor) -> otm bf16
    lag 4: 4 out-transposes [PE] -> ps_ot [128,512]
    lag 5: evac (DVE half + ACT half) -> onat, out DMA [ACT]
  Host converts x to bf16 and the bf16 output back to fp32.
"""

import numpy as np
import ml_dtypes

B, C, T = 16, 512, 4096
NCORES = 8
NCH = C // NCORES          # 64 channels per core
NBLK = T // 128            # 32 blocks of 128 per batch
NU = T // 256              # 16 chunks per batch
PACKW = 392                # cols: A_e 128 | A_o 128 | {B rows0:32 | C_e,C_o rows96:128} 128 | pad

BF16 = ml_dtypes.bfloat16


def _pack_weights(causal_w, causal_b, chunk_w, chunk_b, conv_scale):
    """Build (C, 128, PACKW) bf16 stationary operands + (C, 128, 2) f32 biases."""
    w1 = np.asarray(causal_w, np.float32)[:, 0, :]     # (C,16)
    b1 = np.asarray(causal_b, np.float32)              # (C,)
    w2 = np.asarray(chunk_w, np.float32)[:, 0, :]      # (C,31)
    b2 = np.asarray(chunk_b, np.float32)               # (C,)
    cs = np.asarray(conv_scale, np.float32)            # (2,C,31)

    scale = np.ones((C, 256), np.float32)
    scale[:, :31] += cs[0]
    scale[:, 225:] += cs[1]

    k = np.arange(128)[:, None]
    m = np.arange(128)[None, :]
    d = k - m + 15
    band2 = (d >= 0) & (d <= 30)
    band1 = (d >= 0) & (d <= 15)
    d2 = np.clip(d, 0, 30)
    d1 = np.clip(d, 0, 15)

    w2d = w2[:, d2] * band2            # (C,128,128)
    w1d = w1[:, d1] * band1
    A_e = w2d * scale[:, None, :128] + w1d
    A_o = w2d * scale[:, None, 128:] + w1d

    kc = np.arange(32)[:, None]
    mc = np.arange(32)[None, :]
    kB = np.arange(32)[:, None]
    mB_ = np.arange(128)[None, :]
    dB = kB + 143 - mB_                # B (next-block -> even): w2 taps, k<15, m>=113
    mB = (dB >= 0) & (dB <= 30)
    Bw = w2[:, np.clip(dB, 0, 30)] * mB            # (C,32,128)
    dC = kc - mc - 17                  # C corners: taps 0..15 / 0..14
    mC1 = (dC >= 0) & (dC <= 15)
    Ce_t = w1[:, np.clip(dC, 0, 15)] * mC1
    Co_t = (w2[:, np.clip(dC, 0, 30)] + w1[:, np.clip(dC, 0, 15)]) * mC1

    pack = np.zeros((C, 128, PACKW), np.float32)
    pack[:, :, 0:128] = A_e
    pack[:, :, 128:256] = A_o
    # B lives in rows 0:32 of cols 256:384 (nonzero only cols 369:384);
    # C_e / C_o live in rows 96:128 of cols 256:288 / 288:320 (disjoint rows).
    pack[:, 0:32, 256:384] = Bw
    pack[:, 96:128, 256:288] = Ce_t
    pack[:, 96:128, 288:320] = Co_t

    bias = np.empty((C, 128, 2), np.float32)
    bias[:, :, 0] = scale[:, :128] * b2[:, None] + b1[:, None]   # bias_e
    bias[:, :, 1] = scale[:, 128:] * b2[:, None] + b1[:, None]   # bias_o
    return pack.astype(BF16), bias


def build_nc(nch=NCH, enable_asserts=False, loop_reps=1, skip=()):
    """Build the per-core Bass program (same NEFF for all cores)."""
    import concourse.bacc as bacc
    import concourse.mybir as mybir
    import concourse.tile as tile
    from concourse.ap import AP as BassAP

    fp32 = mybir.dt.float32
    bf16 = mybir.dt.bfloat16
    COPY = mybir.ActivationFunctionType.Identity

    nc = bacc.Bacc("TRN2", target_bir_lowering=False, debug=False,
                   enable_asserts=enable_asserts)

    x_d = nc.dram_tensor("x", [B, nch, T], bf16, kind="ExternalInput").ap()
    w_d = nc.dram_tensor("wpack", [nch, 128, PACKW], bf16, kind="ExternalInput").ap()
    bias_d = nc.dram_tensor("bias", [nch, 128, 2], fp32, kind="ExternalInput").ap()
    id_d = nc.dram_tensor("ident", [128, 128], bf16, kind="ExternalInput").ap()
    o_d = nc.dram_tensor("out", [B, nch, T], bf16, kind="ExternalOutput").ap()

    # DRAM views: per channel [b16, q8, t512] <-> sbuf [128=(b,q), 512]; 1KiB runs
    x_v = x_d.rearrange("b c (q t) -> c b q t", q=8)
    o_v = o_d.rearrange("b c (q t) -> c b q t", q=8)
    # weights: chunks of channels
    WCH = min(8, nch)  # channels per weight DMA
    w_v = w_d.rearrange("(cc c) p w -> cc p c w", cc=nch // WCH)
    bias_v = bias_d.rearrange("c p two -> p c two")

    with tile.TileContext(nc) as tc:
        with (
            tc.tile_pool(name="wbuf", bufs=1) as wbuf_pool,
            tc.tile_pool(name="ident", bufs=1) as id_pool,
            tc.tile_pool(name="xnat", bufs=3) as xnat_pool,
            tc.tile_pool(name="xtm", bufs=1) as xtm_pool,
            tc.tile_pool(name="otm", bufs=3) as otm_pool,
            tc.tile_pool(name="onat", bufs=3) as onat_pool,
            tc.tile_pool(name="ps_it", bufs=2, space="PSUM") as psit_pool,
            tc.tile_pool(name="ps_conv", bufs=3, space="PSUM") as psconv_pool,
            tc.tile_pool(name="ps_ot", bufs=2, space="PSUM") as psot_pool,
        ):
            wbuf = wbuf_pool.tile([128, nch, PACKW], bf16)
            biasbuf = wbuf_pool.tile([128, nch, 2], fp32, tag="biasbuf")
            ident = id_pool.tile([128, 128], bf16)
            nc.gpsimd.dma_start(ident[:], id_d)
            nc.gpsimd.dma_start(biasbuf[:], bias_v)
            for i in range(nch // WCH):
                nc.gpsimd.dma_start(wbuf[:, i * WCH:(i + 1) * WCH, :], w_v[i])

            # Two persistent xtm buffers (alternated by channel parity) so the
            # q9=0 zero columns are memset once, not per channel.
            xtm0 = xtm_pool.tile([128, 576], bf16, tag="xtm0", name="xtm0")
            xtm1 = xtm_pool.tile([128, 576], bf16, tag="xtm1", name="xtm1")
            xtms = [xtm0, xtm1]
            for xt in xtms:
                xvz = xt[:].rearrange("p (jj two b q9) -> p jj two b q9",
                                      jj=2, two=2, b=16, q9=9)
                nc.vector.memset(xvz[:, :, :, :, 0], 0.0)

            # Per-channel pipeline state (tiles passed between stages).
            xnats = [None] * nch
            psits = [None] * nch
            psconvs = [None] * nch
            otms = [None] * nch
            psots = [None] * nch
            onats = [None] * nch

            def st_indma(c):
                xnats[c] = xnat_pool.tile([128, 512], bf16, tag="xnat",
                                          name="xnat")
                nc.sync.dma_start(xnats[c][:], x_v[c])

            def st_intrans(c):
                ps_it = psit_pool.tile([128, 512], fp32, tag="ps_it",
                                       name="ps_it")
                psits[c] = ps_it
                for j in range(4):
                    nc.tensor.matmul(
                        ps_it[:, j * 128:(j + 1) * 128],
                        xnats[c][:, j * 128:(j + 1) * 128],
                        ident[:],
                        start=True, stop=True, skip_group_check=True)
                xtm = xtms[c % 2]
                xv = xtm[:].rearrange("p (jj two b q9) -> p jj two b q9",
                                      jj=2, two=2, b=16, q9=9)
                nc.vector.tensor_copy(
                    xv[:, :, :, :, 1:9],
                    ps_it[:].rearrange("p (jj two b q) -> p jj two b q",
                                       jj=2, two=2, b=16))

            def st_conv(c):
                xtm = xtms[c % 2]
                xv = xtm[:].rearrange("p (jj two b q9) -> p jj two b q9",
                                      jj=2, two=2, b=16, q9=9)
                # psum cols ordered (up, b, uh): u = 2*uh + up.
                rhs_even = xv[:, :, 0, :, 1:9]    # block 2u   = (jj=up, two=0)
                rhs_odd = xv[:, :, 1, :, 1:9]     # block 2u+1 = (jj=up, two=1)
                wA_e = wbuf[:, c, 0:128]
                wA_o = wbuf[:, c, 128:256]
                wB = wbuf[0:32, c, 256:384]
                wCe = wbuf[:, c, 256:288]
                wCo = wbuf[:, c, 288:320]

                ps_e = psconv_pool.tile([128, 256], fp32, tag="ps_conv",
                                        name="ps_e")
                ps_o = psconv_pool.tile([128, 256], fp32, tag="ps_conv",
                                        name="ps_o")
                psconvs[c] = (ps_e, ps_o)
                nc.tensor.matmul(ps_e[:], wA_e, rhs_even,
                                 start=True, stop=False, skip_group_check=True)
                nc.tensor.matmul(ps_e[:], wB, xv[0:32, :, 1, :, 1:9],
                                 start=False, stop=False, skip_group_check=True)
                # causal carry rhs, cols (up, b, uh):
                # col = 432 - 287*up + 9b + uh; up=0 hits the j=3 segment
                # (q9=0 -> zero col), up=1 the j=1 segment.
                rhs_prev = BassAP(tensor=xtm[:].tensor, offset=432,
                                  ap=[[576, 128], [-287, 2], [9, 16], [1, 8]])
                nc.tensor.matmul(ps_e[0:32, :], wCe, rhs_prev,
                                 start=False, stop=True, skip_group_check=True)

                nc.tensor.matmul(ps_o[:], wA_o, rhs_odd,
                                 start=True, stop=False, skip_group_check=True)
                nc.tensor.matmul(ps_o[0:32, :], wCo, rhs_even,
                                 start=False, stop=True, skip_group_check=True)

            def st_act(c):
                ps_e, ps_o = psconvs[c]
                otm_e = otm_pool.tile([128, 256], bf16, tag="otm", name="otm_e")
                otm_o = otm_pool.tile([128, 256], bf16, tag="otm", name="otm_o")
                otms[c] = (otm_e, otm_o)
                nc.scalar.activation(otm_e[:], ps_e[:], COPY,
                                     bias=biasbuf[:, c, 0:1])
                nc.scalar.activation(otm_o[:], ps_o[:], COPY,
                                     bias=biasbuf[:, c, 1:2])
                psconvs[c] = None

            def st_outtrans(c):
                otm_e, otm_o = otms[c]
                ps_ot = psot_pool.tile([128, 512], fp32, tag="ps_ot",
                                       name="ps_ot")
                psots[c] = ps_ot
                for half in range(2):
                    for par, otm in ((0, otm_e), (1, otm_o)):
                        nc.tensor.matmul(
                            ps_ot[:, half * 256 + par * 128:
                                  half * 256 + (par + 1) * 128],
                            otm[:, half * 128:(half + 1) * 128],
                            ident[:],
                            start=True, stop=True, skip_group_check=True)
                otms[c] = None

            def st_out(c):
                ps_ot = psots[c]
                onat = onat_pool.tile([128, 512], bf16, tag="onat",
                                      name="onat")
                onats[c] = onat
                nc.vector.tensor_copy(onat[:, 0:256], ps_ot[:, 0:256])
                nc.scalar.activation(onat[:, 256:512], ps_ot[:, 256:512],
                                     mybir.ActivationFunctionType.Copy,
                                     bias=0.0)
                nc.scalar.dma_start(o_v[c], onat[:])
                psots[c] = None

            import contextlib
            loop_cm = (tc.For_i(0, loop_reps, 1) if loop_reps > 1
                       else contextlib.nullcontext())
            with loop_cm:
              for i in range(nch + 5):
                  if i < nch and "indma" not in skip:
                      st_indma(i)
                  if 0 <= i - 1 < nch and "intrans" not in skip:
                      st_intrans(i - 1)
                  if 0 <= i - 2 < nch and "conv" not in skip:
                      st_conv(i - 2)
                  if 0 <= i - 3 < nch and "act" not in skip:
                      st_act(i - 3)
                  if 0 <= i - 4 < nch and "outtrans" not in skip:
                      st_outtrans(i - 4)
                  if 0 <= i - 5 < nch and "outdma" not in skip:
                      st_out(i - 5)

    nc.compile()
    return nc


def make_core_inputs(x, causal_w, causal_b, chunk_w, chunk_b, conv_scale):
    """Shard host-side inputs for the 8 cores (bf16 conversion included)."""
    x = np.asarray(x, np.float32).astype(BF16)
    pack, bias = _pack_weights(causal_w, causal_b, chunk_w, chunk_b, conv_scale)
    ident = np.eye(128, dtype=BF16)
    in_maps = []
    for i in range(NCORES):
        sl = slice(i * NCH, (i + 1) * NCH)
        in_maps.append({
            "x": np.ascontiguousarray(x[:, sl, :]),
            "wpack": np.ascontiguousarray(pack[sl]),
            "bias": np.ascontiguousarray(bias[sl]),
            "ident": ident,
        })
    return in_maps


def kernel(x, causal_w, causal_b, chunk_w, chunk_b, conv_scale, chunk_size):
    from concourse.bass_utils import run_bass_kernel_spmd

    assert int(chunk_size) == 256
    in_maps = make_core_inputs(x, causal_w, causal_b, chunk_w, chunk_b,
                               conv_scale)
    nc = build_nc()
    core_ids = list(range(NCORES))
    res = run_bass_kernel_spmd(nc, in_maps, core_ids)
    out = np.empty((B, C, T), np.float32)
    for i in core_ids:
        out[:, i * NCH:(i + 1) * NCH, :] = res.results[i]["out"].astype(np.float32)
    return out


# revision 17
# speedup vs baseline: 12.6878x; 1.5967x over previous
"""Trainium2 Bass kernel for ChunkCausalDepthwiseConv1d (bf16, SW-pipelined).

Problem: x (16, 512, 4096) f32; per-channel depthwise convs:
  out = chunk_scale * (chunkconv_K31_same_per_256chunk(x) + chunk_b)
        + causalconv_K16(x) + causal_b

Strategy (8 NeuronCores, channel-parallel, 64 ch/core, all batches),
everything bf16 on device (tolerance 2e-2 rel L2; bf16 lands ~3e-3):
  Per-channel Toeplitz matmuls on the TensorEngine in a time-major
  (transposed) domain. All transposes run as REGULAR matmuls (lhsT =
  data block, rhs = identity): they pipeline as LDW+MM pairs (~81ns vs
  ~275ns for transpose-mode) and keep the PE HAM clock at 2.4 GHz.
  bf16 halves DMA (x 8 MiB + wpack 6.4 MiB + out 8 MiB per core) and
  enables fast-weight-load on the 128-col stationaries.

  The 8 per-channel stages are SOFTWARE-PIPELINED across channels with
  explicit lags so every engine's in-order queue always holds ready
  work (without this, the PE serializes each channel's full
  cross-engine round trip: measured 3.1us/ch vs ~1.3us of PE work):
    lag 0: x DMA          [SP]    xnat [128=(b,q), 512t]
    lag 1: 4 in-transposes [PE] -> ps_it [128,512], 1 evac [DVE] ->
           xtm[t, j*144+b*9+q9] (q9=0 cols zero; memset hoisted into
           two persistent buffers alternated by channel parity)
    lag 2: 5 conv matmuls  [PE] -> ps_e/ps_o [128,256], cols (u%2,b,u//2)
    lag 3: 2 Identity+bias [ACT] (fp32 bias tensor) -> otm bf16
    lag 4: 4 out-transposes [PE] -> ps_ot [128,512]
    lag 5: evac (DVE half + ACT half) -> onat, out DMA [ACT]
  Host converts x to bf16 and the bf16 output back to fp32.
"""

import numpy as np
import ml_dtypes

B, C, T = 16, 512, 4096
NCORES = 8
NCH = C // NCORES          # 64 channels per core
NBLK = T // 128            # 32 blocks of 128 per batch
NU = T // 256              # 16 chunks per batch
PACKW = 392                # cols: A_e 128 | A_o 128 | {B rows0:32 | C_e,C_o rows96:128} 128 | pad

BF16 = ml_dtypes.bfloat16


def _pack_weights(causal_w, causal_b, chunk_w, chunk_b, conv_scale):
    """Build (C, 128, PACKW) bf16 stationary operands + (C, 128, 2) f32 biases."""
    w1 = np.asarray(causal_w, np.float32)[:, 0, :]     # (C,16)
    b1 = np.asarray(causal_b, np.float32)              # (C,)
    w2 = np.asarray(chunk_w, np.float32)[:, 0, :]      # (C,31)
    b2 = np.asarray(chunk_b, np.float32)               # (C,)
    cs = np.asarray(conv_scale, np.float32)            # (2,C,31)

    scale = np.ones((C, 256), np.float32)
    scale[:, :31] += cs[0]
    scale[:, 225:] += cs[1]

    k = np.arange(128)[:, None]
    m = np.arange(128)[None, :]
    d = k - m + 15
    band2 = (d >= 0) & (d <= 30)
    band1 = (d >= 0) & (d <= 15)
    d2 = np.clip(d, 0, 30)
    d1 = np.clip(d, 0, 15)

    w2d = w2[:, d2] * band2            # (C,128,128)
    w1d = w1[:, d1] * band1
    A_e = w2d * scale[:, None, :128] + w1d
    A_o = w2d * scale[:, None, 128:] + w1d

    kc = np.arange(32)[:, None]
    mc = np.arange(32)[None, :]
    kB = np.arange(32)[:, None]
    mB_ = np.arange(128)[None, :]
    dB = kB + 143 - mB_                # B (next-block -> even): w2 taps, k<15, m>=113
    mB = (dB >= 0) & (dB <= 30)
    Bw = w2[:, np.clip(dB, 0, 30)] * mB            # (C,32,128)
    dC = kc - mc - 17                  # C corners: taps 0..15 / 0..14
    mC1 = (dC >= 0) & (dC <= 15)
    Ce_t = w1[:, np.clip(dC, 0, 15)] * mC1
    Co_t = (w2[:, np.clip(dC, 0, 30)] + w1[:, np.clip(dC, 0, 15)]) * mC1

    pack = np.zeros((C, 128, PACKW), np.float32)
    pack[:, :, 0:128] = A_e
    pack[:, :, 128:256] = A_o
    # B lives in rows 0:32 of cols 256:384 (nonzero only cols 369:384);
    # C_e / C_o live in rows 96:128 of cols 256:288 / 288:320 (disjoint rows).
    pack[:, 0:32, 256:384] = Bw
    pack[:, 96:128, 256:288] = Ce_t
    pack[:, 96:128, 288:320] = Co_t

    bias = np.empty((C, 128, 2), np.float32)
    bias[:, :, 0] = scale[:, :128] * b2[:, None] + b1[:, None]   # bias_e
    bias[:, :, 1] = scale[:, 128:] * b2[:, None] + b1[:, None]   # bias_o
    return pack.astype(BF16), bias


def build_nc(nch=NCH, enable_asserts=False, loop_reps=1, skip=()):
    """Build the per-core Bass program (same NEFF for all cores)."""
    import concourse.bacc as bacc
    import concourse.mybir as mybir
    import concourse.tile as tile
    from concourse.ap import AP as BassAP

    fp32 = mybir.dt.float32
    bf16 = mybir.dt.bfloat16
    COPY = mybir.ActivationFunctionType.Identity

    nc = bacc.Bacc("TRN2", target_bir_lowering=False, debug=False,
                   enable_asserts=enable_asserts)

    x_d = nc.dram_tensor("x", [nch, B, T], bf16, kind="ExternalInput").ap()
    w_d = nc.dram_tensor("wpack", [nch, 128, PACKW], bf16, kind="ExternalInput").ap()
    bias_d = nc.dram_tensor("bias", [nch, 128, 2], fp32, kind="ExternalInput").ap()
    id_d = nc.dram_tensor("ident", [128, 128], bf16, kind="ExternalInput").ap()
    o_d = nc.dram_tensor("out", [nch, B, T], bf16, kind="ExternalOutput").ap()

    # DRAM views: x/out are CHANNEL-MAJOR [nch, B, T] (host transposes) so
    # that (b,q) merges into one contiguous 128-count stride-512 dim and 8
    # channels batch into ONE DMA (each dma_start costs ~600ns of
    # issuing-engine sequencer time): sbuf [128=(b,q), c8, t512]; 1KiB runs.
    CG = 8  # channels per DMA group
    x_v = x_d.rearrange("(g c) b (q t) -> g (b q) c t", c=CG, t=512)
    o_v = o_d.rearrange("(g c) b (q t) -> g (b q) c t", c=CG, t=512)
    # weights: chunks of channels
    WCH = min(8, nch)  # channels per weight DMA
    w_v = w_d.rearrange("(cc c) p w -> cc p c w", cc=nch // WCH)
    bias_v = bias_d.rearrange("c p two -> p c two")

    with tile.TileContext(nc) as tc:
        with (
            tc.tile_pool(name="wbuf", bufs=1) as wbuf_pool,
            tc.tile_pool(name="ident", bufs=1) as id_pool,
            tc.tile_pool(name="xnat", bufs=3) as xnat_pool,
            tc.tile_pool(name="xtm", bufs=1) as xtm_pool,
            tc.tile_pool(name="otm", bufs=3) as otm_pool,
            tc.tile_pool(name="onat", bufs=2) as onat_pool,
            tc.tile_pool(name="ps_it", bufs=2, space="PSUM") as psit_pool,
            tc.tile_pool(name="ps_conv", bufs=3, space="PSUM") as psconv_pool,
            tc.tile_pool(name="ps_ot", bufs=2, space="PSUM") as psot_pool,
        ):
            wbuf = wbuf_pool.tile([128, nch, PACKW], bf16)
            biasbuf = wbuf_pool.tile([128, nch, 2], fp32, tag="biasbuf")
            ident = id_pool.tile([128, 128], bf16)
            nc.gpsimd.dma_start(ident[:], id_d)
            nc.gpsimd.dma_start(biasbuf[:], bias_v)
            for i in range(nch // WCH):
                nc.gpsimd.dma_start(wbuf[:, i * WCH:(i + 1) * WCH, :], w_v[i])

            # Two persistent xtm buffers (alternated by channel parity) so the
            # q9=0 zero columns are memset once, not per channel.
            xtm0 = xtm_pool.tile([128, 576], bf16, tag="xtm0", name="xtm0")
            xtm1 = xtm_pool.tile([128, 576], bf16, tag="xtm1", name="xtm1")
            xtms = [xtm0, xtm1]
            for xt in xtms:
                xvz = xt[:].rearrange("p (jj two b q9) -> p jj two b q9",
                                      jj=2, two=2, b=16, q9=9)
                nc.vector.memset(xvz[:, :, :, :, 0], 0.0)

            # Per-channel pipeline state (tiles passed between stages).
            ngrp = nch // CG
            xnats = [None] * ngrp
            psits = [None] * nch
            psconvs = [None] * nch
            otms = [None] * nch
            psots = [None] * nch
            onats = [None] * ngrp

            def st_indma(g):
                xnats[g] = xnat_pool.tile([128, CG, 512], bf16, tag="xnat",
                                          name="xnat")
                nc.sync.dma_start(xnats[g][:], x_v[g])

            def st_intrans(c):
                ps_it = psit_pool.tile([128, 512], fp32, tag="ps_it",
                                       name="ps_it")
                psits[c] = ps_it
                xn = xnats[c // CG][:, c % CG, :]
                for j in range(4):
                    nc.tensor.matmul(
                        ps_it[:, j * 128:(j + 1) * 128],
                        xn[:, j * 128:(j + 1) * 128],
                        ident[:],
                        start=True, stop=True, skip_group_check=True)
                xtm = xtms[c % 2]
                xv = xtm[:].rearrange("p (jj two b q9) -> p jj two b q9",
                                      jj=2, two=2, b=16, q9=9)
                nc.vector.tensor_copy(
                    xv[:, :, :, :, 1:9],
                    ps_it[:].rearrange("p (jj two b q) -> p jj two b q",
                                       jj=2, two=2, b=16))

            def st_conv(c):
                xtm = xtms[c % 2]
                xv = xtm[:].rearrange("p (jj two b q9) -> p jj two b q9",
                                      jj=2, two=2, b=16, q9=9)
                # psum cols ordered (up, b, uh): u = 2*uh + up.
                rhs_even = xv[:, :, 0, :, 1:9]    # block 2u   = (jj=up, two=0)
                rhs_odd = xv[:, :, 1, :, 1:9]     # block 2u+1 = (jj=up, two=1)
                wA_e = wbuf[:, c, 0:128]
                wA_o = wbuf[:, c, 128:256]
                wB = wbuf[0:32, c, 256:384]
                wCe = wbuf[:, c, 256:288]
                wCo = wbuf[:, c, 288:320]

                ps_e = psconv_pool.tile([128, 256], fp32, tag="ps_conv",
                                        name="ps_e")
                ps_o = psconv_pool.tile([128, 256], fp32, tag="ps_conv",
                                        name="ps_o")
                psconvs[c] = (ps_e, ps_o)
                nc.tensor.matmul(ps_e[:], wA_e, rhs_even,
                                 start=True, stop=False, skip_group_check=True)
                nc.tensor.matmul(ps_e[:], wB, xv[0:32, :, 1, :, 1:9],
                                 start=False, stop=False, skip_group_check=True)
                # causal carry rhs, cols (up, b, uh):
                # col = 432 - 287*up + 9b + uh; up=0 hits the j=3 segment
                # (q9=0 -> zero col), up=1 the j=1 segment.
                rhs_prev = BassAP(tensor=xtm[:].tensor, offset=432,
                                  ap=[[576, 128], [-287, 2], [9, 16], [1, 8]])
                nc.tensor.matmul(ps_e[0:32, :], wCe, rhs_prev,
                                 start=False, stop=True, skip_group_check=True)

                nc.tensor.matmul(ps_o[:], wA_o, rhs_odd,
                                 start=True, stop=False, skip_group_check=True)
                nc.tensor.matmul(ps_o[0:32, :], wCo, rhs_even,
                                 start=False, stop=True, skip_group_check=True)

            def st_act(c):
                ps_e, ps_o = psconvs[c]
                otm_e = otm_pool.tile([128, 256], bf16, tag="otm", name="otm_e")
                otm_o = otm_pool.tile([128, 256], bf16, tag="otm", name="otm_o")
                otms[c] = (otm_e, otm_o)
                nc.scalar.activation(otm_e[:], ps_e[:], COPY,
                                     bias=biasbuf[:, c, 0:1])
                nc.scalar.activation(otm_o[:], ps_o[:], COPY,
                                     bias=biasbuf[:, c, 1:2])
                psconvs[c] = None

            def st_outtrans(c):
                otm_e, otm_o = otms[c]
                ps_ot = psot_pool.tile([128, 512], fp32, tag="ps_ot",
                                       name="ps_ot")
                psots[c] = ps_ot
                for half in range(2):
                    for par, otm in ((0, otm_e), (1, otm_o)):
                        nc.tensor.matmul(
                            ps_ot[:, half * 256 + par * 128:
                                  half * 256 + (par + 1) * 128],
                            otm[:, half * 128:(half + 1) * 128],
                            ident[:],
                            start=True, stop=True, skip_group_check=True)
                otms[c] = None

            def st_out(c):
                ps_ot = psots[c]
                g, cc = c // CG, c % CG
                if cc == 0:
                    onats[g] = onat_pool.tile([128, CG, 512], bf16,
                                              tag="onat", name="onat")
                onat = onats[g]
                nc.vector.tensor_copy(onat[:, cc, 0:256], ps_ot[:, 0:256])
                nc.scalar.activation(onat[:, cc, 256:512], ps_ot[:, 256:512],
                                     mybir.ActivationFunctionType.Copy,
                                     bias=0.0)
                if cc == CG - 1:
                    nc.scalar.dma_start(o_v[g], onats[g][:])
                psots[c] = None

            import contextlib
            loop_cm = (tc.For_i(0, loop_reps, 1) if loop_reps > 1
                       else contextlib.nullcontext())
            with loop_cm:
              for i in range(nch + 5):
                  # prefetch x one group (CG iterations) ahead of its stage
                  if "indma" not in skip:
                      if i == 0:
                          st_indma(0)
                          if ngrp > 1:
                              st_indma(1)
                      elif i % CG == 0 and i // CG + 1 < ngrp:
                          st_indma(i // CG + 1)
                  if 0 <= i - 1 < nch and "intrans" not in skip:
                      st_intrans(i - 1)
                  if 0 <= i - 2 < nch and "conv" not in skip:
                      st_conv(i - 2)
                  if 0 <= i - 3 < nch and "act" not in skip:
                      st_act(i - 3)
                  if 0 <= i - 4 < nch and "outtrans" not in skip:
                      st_outtrans(i - 4)
                  if 0 <= i - 5 < nch and "outdma" not in skip:
                      st_out(i - 5)

    nc.compile()
    return nc


def make_core_inputs(x, causal_w, causal_b, chunk_w, chunk_b, conv_scale):
    """Shard host-side inputs for the 8 cores (bf16 conversion included)."""
    x = np.asarray(x, np.float32).astype(BF16).transpose(1, 0, 2)  # [C, B, T]
    pack, bias = _pack_weights(causal_w, causal_b, chunk_w, chunk_b, conv_scale)
    ident = np.eye(128, dtype=BF16)
    in_maps = []
    for i in range(NCORES):
        sl = slice(i * NCH, (i + 1) * NCH)
        in_maps.append({
            "x": np.ascontiguousarray(x[sl]),
            "wpack": np.ascontiguousarray(pack[sl]),
            "bias": np.ascontiguousarray(bias[sl]),
            "ident": ident,
        })
    return in_maps


def kernel(x, causal_w, causal_b, chunk_w, chunk_b, conv_scale, chunk_size):
    from concourse.bass_utils import run_bass_kernel_spmd

    assert int(chunk_size) == 256
    in_maps = make_core_inputs(x, causal_w, causal_b, chunk_w, chunk_b,
                               conv_scale)
    nc = build_nc()
    core_ids = list(range(NCORES))
    res = run_bass_kernel_spmd(nc, in_maps, core_ids)
    out = np.empty((B, C, T), np.float32)
    for i in core_ids:
        # device output is channel-major [NCH, B, T]
        out[:, i * NCH:(i + 1) * NCH, :] = (
            res.results[i]["out"].astype(np.float32).transpose(1, 0, 2))
    return out


# revision 19
# speedup vs baseline: 13.7283x; 1.0820x over previous
"""Trainium2 Bass kernel for ChunkCausalDepthwiseConv1d (bf16, minimal device).

Problem: x (16, 512, 4096) f32; per-channel depthwise convs:
  out = chunk_scale * (chunkconv_K31_same_per_256chunk(x) + chunk_b)
        + causalconv_K16(x) + causal_b

Strategy (8 NeuronCores, channel-parallel, 64 ch/core, all batches),
everything bf16 on device (tolerance 2e-2 rel L2; bf16 lands ~3e-3).
The conv is cast as per-channel Toeplitz matmuls in a TIME-MAJOR
domain; the layout permutations in and out are folded into the host's
shard/unshard step, so the device does ONLY:
  per group of 8 channels: 1 batched in-DMA; per channel: 5 bf16
  matmuls -> psum_e/psum_o [128, 256] fp32 + 2 Identity+bias
  evacuations (DVE / ACT, fp32 bias) -> otm bf16; 1 batched out-DMA.

  - host uploads x TIME-MAJOR: xup[c, p, col] with col = j*144+b*9+q9,
    j = (block k = 4q+j within batch) mod 4... precisely: block k of
    batch b (t = 512q + 128j + p) sits at col (j, b, q9=q+1); q9=0
    cols are ZEROS (the causal carry for each batch's first chunk,
    addressed by the prev-block rhs AP below).
  - psum cols ordered (up, b, uh), u = 2*uh + up (chunk index):
      ps_e: A_e @ even-blocks + B @ odd-heads + C_e @ prev-odd-tails
      ps_o: A_o @ odd-blocks + C_o @ even-tails
    with per-channel stationaries (chunk_scale folded into A/B) packed
    on the host exactly as in earlier revisions.
  - device output is time-major otm[c, p, par*256 + up*128 + b*8 + uh]
    = out[b, c, 256*(2uh+up) + 128*par + p]; the host inverts this
    permutation during unshard (pure data movement, no arithmetic).
  - 8-channel DMA groups: each dma_start costs ~600ns of
    issuing-engine sequencer time, so batch them.
"""

import numpy as np
import ml_dtypes

B, C, T = 16, 512, 4096
NCORES = 8
NCH = C // NCORES          # 64 channels per core
NU = T // 256              # 16 chunks per batch
PACKW = 392                # cols: A_e 128 | A_o 128 | {B rows0:32 | C_e,C_o rows96:128} 128 | pad
CG = 8                     # channels per DMA group

BF16 = ml_dtypes.bfloat16


def _pack_weights(causal_w, causal_b, chunk_w, chunk_b, conv_scale):
    """Build (C, 128, PACKW) bf16 stationary operands + (C, 128, 2) f32 biases."""
    w1 = np.asarray(causal_w, np.float32)[:, 0, :]     # (C,16)
    b1 = np.asarray(causal_b, np.float32)              # (C,)
    w2 = np.asarray(chunk_w, np.float32)[:, 0, :]      # (C,31)
    b2 = np.asarray(chunk_b, np.float32)               # (C,)
    cs = np.asarray(conv_scale, np.float32)            # (2,C,31)

    scale = np.ones((C, 256), np.float32)
    scale[:, :31] += cs[0]
    scale[:, 225:] += cs[1]

    k = np.arange(128)[:, None]
    m = np.arange(128)[None, :]
    d = k - m + 15
    band2 = (d >= 0) & (d <= 30)
    band1 = (d >= 0) & (d <= 15)
    d2 = np.clip(d, 0, 30)
    d1 = np.clip(d, 0, 15)

    w2d = w2[:, d2] * band2            # (C,128,128)
    w1d = w1[:, d1] * band1
    A_e = w2d * scale[:, None, :128] + w1d
    A_o = w2d * scale[:, None, 128:] + w1d

    kc = np.arange(32)[:, None]
    mc = np.arange(32)[None, :]
    kB = np.arange(32)[:, None]
    mB_ = np.arange(128)[None, :]
    dB = kB + 143 - mB_                # B (next-block -> even): w2 taps, k<15, m>=113
    mB = (dB >= 0) & (dB <= 30)
    Bw = w2[:, np.clip(dB, 0, 30)] * mB            # (C,32,128)
    dC = kc - mc - 17                  # C corners: taps 0..15 / 0..14
    mC1 = (dC >= 0) & (dC <= 15)
    Ce_t = w1[:, np.clip(dC, 0, 15)] * mC1
    Co_t = (w2[:, np.clip(dC, 0, 30)] + w1[:, np.clip(dC, 0, 15)]) * mC1

    pack = np.zeros((C, 128, PACKW), np.float32)
    pack[:, :, 0:128] = A_e
    pack[:, :, 128:256] = A_o
    # B lives in rows 0:32 of cols 256:384 (nonzero only cols 369:384);
    # C_e / C_o live in rows 96:128 of cols 256:288 / 288:320 (disjoint rows).
    pack[:, 0:32, 256:384] = Bw
    pack[:, 96:128, 256:288] = Ce_t
    pack[:, 96:128, 288:320] = Co_t

    bias = np.empty((C, 128, 2), np.float32)
    bias[:, :, 0] = scale[:, :128] * b2[:, None] + b1[:, None]   # bias_e
    bias[:, :, 1] = scale[:, 128:] * b2[:, None] + b1[:, None]   # bias_o
    return pack.astype(BF16), bias


def build_nc(nch=NCH, enable_asserts=False, loop_reps=1, skip=()):
    """Build the per-core Bass program (same NEFF for all cores)."""
    import concourse.bacc as bacc
    import concourse.mybir as mybir
    import concourse.tile as tile
    from concourse.ap import AP as BassAP

    fp32 = mybir.dt.float32
    bf16 = mybir.dt.bfloat16
    COPY = mybir.ActivationFunctionType.Identity

    nc = bacc.Bacc("TRN2", target_bir_lowering=False, debug=False,
                   enable_asserts=enable_asserts)

    x_d = nc.dram_tensor("x", [nch, 128, 576], bf16, kind="ExternalInput").ap()
    w_d = nc.dram_tensor("wpack", [nch, 128, PACKW], bf16, kind="ExternalInput").ap()
    bias_d = nc.dram_tensor("bias", [nch, 128, 2], fp32, kind="ExternalInput").ap()
    o_d = nc.dram_tensor("out", [nch, 128, 512], bf16, kind="ExternalOutput").ap()

    ngrp = nch // CG
    x_v = x_d.rearrange("(g c) p w -> g p c w", c=CG)
    o_v = o_d.rearrange("(g c) p w -> g p c w", c=CG)
    WCH = min(8, nch)  # channels per weight DMA
    w_v = w_d.rearrange("(cc c) p w -> cc p c w", cc=nch // WCH)
    bias_v = bias_d.rearrange("c p two -> p c two")

    with tile.TileContext(nc) as tc:
        with (
            tc.tile_pool(name="wbuf", bufs=1) as wbuf_pool,
            tc.tile_pool(name="xtm", bufs=3) as xtm_pool,
            tc.tile_pool(name="otm", bufs=2) as otm_pool,
            tc.tile_pool(name="ps_conv", bufs=4, space="PSUM") as psconv_pool,
        ):
            wbuf = wbuf_pool.tile([128, nch, PACKW], bf16)
            biasbuf = wbuf_pool.tile([128, nch, 2], fp32, tag="biasbuf")
            nc.gpsimd.dma_start(biasbuf[:], bias_v)
            for i in range(nch // WCH):
                nc.gpsimd.dma_start(wbuf[:, i * WCH:(i + 1) * WCH, :], w_v[i])

            xtms = [None] * ngrp
            otms = [None] * ngrp
            psconvs = [None] * nch

            def st_indma(g):
                xtms[g] = xtm_pool.tile([128, CG, 576], bf16, tag="xtm",
                                        name="xtm")
                nc.sync.dma_start(xtms[g][:], x_v[g])

            def st_conv(c):
                xt = xtms[c // CG][:]
                co = (c % CG) * 576
                # rhs cols (up, b, uh): block col = jj*144 + b*9 + q9 with
                # jj = up, q9 = uh+1; two=0 even / two=1 odd (+72 per `two`).
                def rhs(offset, parts=128):
                    return BassAP(tensor=xt.tensor, offset=co + offset,
                                  ap=[[CG * 576, parts], [288, 2], [9, 16],
                                      [1, 8]])
                rhs_even = rhs(1)
                rhs_odd = rhs(145)
                rhs_odd_head = rhs(145, 32)
                # causal carry: col = 432 - 287*up + 9b + uh; up=0 hits the
                # j=3 segment q9=0 (zero col), up=1 the j=1 segment.
                rhs_prev = BassAP(tensor=xt.tensor, offset=co + 432,
                                  ap=[[CG * 576, 128], [-287, 2], [9, 16],
                                      [1, 8]])
                wA_e = wbuf[:, c, 0:128]
                wA_o = wbuf[:, c, 128:256]
                wB = wbuf[0:32, c, 256:384]
                wCe = wbuf[:, c, 256:288]
                wCo = wbuf[:, c, 288:320]

                ps_e = psconv_pool.tile([128, 256], fp32, tag="ps_conv",
                                        name="ps_e")
                ps_o = psconv_pool.tile([128, 256], fp32, tag="ps_conv",
                                        name="ps_o")
                psconvs[c] = (ps_e, ps_o)
                nc.tensor.matmul(ps_e[:], wA_e, rhs_even,
                                 start=True, stop=False, skip_group_check=True)
                nc.tensor.matmul(ps_e[:], wB, rhs_odd_head,
                                 start=False, stop=False, skip_group_check=True)
                nc.tensor.matmul(ps_e[0:32, :], wCe, rhs_prev,
                                 start=False, stop=True, skip_group_check=True)

                nc.tensor.matmul(ps_o[:], wA_o, rhs_odd,
                                 start=True, stop=False, skip_group_check=True)
                nc.tensor.matmul(ps_o[0:32, :], wCo, rhs_even,
                                 start=False, stop=True, skip_group_check=True)

            def st_out(c):
                ps_e, ps_o = psconvs[c]
                g, cc = c // CG, c % CG
                if cc == 0:
                    otms[g] = otm_pool.tile([128, CG, 512], bf16,
                                            tag="otm", name="otm")
                otm = otms[g]
                nc.vector.tensor_scalar_add(otm[:, cc, 0:256], ps_e[:],
                                            biasbuf[:, c, 0:1])
                nc.scalar.activation(otm[:, cc, 256:512], ps_o[:], COPY,
                                     bias=biasbuf[:, c, 1:2])
                if cc == CG - 1:
                    nc.scalar.dma_start(o_v[g], otm[:])
                psconvs[c] = None

            import contextlib
            loop_cm = (tc.For_i(0, loop_reps, 1) if loop_reps > 1
                       else contextlib.nullcontext())
            with loop_cm:
              for i in range(nch + 2):
                  # prefetch x one group (CG iterations) ahead of its stage
                  if "indma" not in skip:
                      if i == 0:
                          st_indma(0)
                          if ngrp > 1:
                              st_indma(1)
                      elif i % CG == 0 and i // CG + 1 < ngrp:
                          st_indma(i // CG + 1)
                  if 0 <= i - 1 < nch and "conv" not in skip:
                      st_conv(i - 1)
                  if 0 <= i - 2 < nch and "outdma" not in skip:
                      st_out(i - 2)

    nc.compile()
    return nc


def make_core_inputs(x, causal_w, causal_b, chunk_w, chunk_b, conv_scale):
    """Shard host-side inputs for the 8 cores.

    x is uploaded TIME-MAJOR with zero carry columns baked in:
      xup[c, p, j*144 + b*9 + (q+1)] = x[b, c, 512q + 128j + p], q9=0 cols 0.
    """
    x = np.asarray(x, np.float32).astype(BF16)
    # [B, C, q8, j4, p128] -> [C, p, j, b, q]
    xr = x.reshape(B, C, 8, 4, 128).transpose(1, 4, 3, 0, 2)
    xup = np.zeros((C, 128, 4, 16, 9), dtype=BF16)
    xup[:, :, :, :, 1:9] = xr
    xup = xup.reshape(C, 128, 576)

    pack, bias = _pack_weights(causal_w, causal_b, chunk_w, chunk_b, conv_scale)
    in_maps = []
    for i in range(NCORES):
        sl = slice(i * NCH, (i + 1) * NCH)
        in_maps.append({
            "x": np.ascontiguousarray(xup[sl]),
            "wpack": np.ascontiguousarray(pack[sl]),
            "bias": np.ascontiguousarray(bias[sl]),
        })
    return in_maps


def _assemble_output(res_outs):
    """Invert the device's time-major layout back to (B, C, T) fp32.

    dev[c, p, par*256 + up*128 + b*8 + uh] = out[b, c, 256*(2uh+up) + 128par + p]
    """
    dev = np.concatenate(res_outs, axis=0)                 # [C, 128, 512]
    dev = dev.reshape(C, 128, 2, 2, 16, 8)                 # [c,p,par,up,b,uh]
    out = dev.transpose(4, 0, 5, 3, 2, 1)                  # [b,c,uh,up,par,p]
    return np.ascontiguousarray(out.reshape(B, C, T)).astype(np.float32)


def kernel(x, causal_w, causal_b, chunk_w, chunk_b, conv_scale, chunk_size):
    from concourse.bass_utils import run_bass_kernel_spmd

    assert int(chunk_size) == 256
    in_maps = make_core_inputs(x, causal_w, causal_b, chunk_w, chunk_b,
                               conv_scale)
    nc = build_nc()
    core_ids = list(range(NCORES))
    res = run_bass_kernel_spmd(nc, in_maps, core_ids)
    return _assemble_output([res.results[i]["out"] for i in core_ids])
